# revision 1
# baseline (speedup 1.0000x reference)
"""Trainium2 Bass kernel for the mu/sigma Conv2d problem.

Math (per reference):
  mu_y    = conv(mu_x, W) + bias
  sigma_y = (softplus(w_sigma) * (conv(sigma_x, ones) + conv(mu_x^2, ones))
             + conv(sigma_x, W^2)) * 1e-3

Shapes: mu_x/sigma_x [16,128,96,96], W [256,128,5,5], bias [256],
w_sigma [256,1].  Outputs [16,256,92,92] (VALID conv).

Strategy: data-parallel over batch across 8 NeuronCores (2 images/core).
Each conv is a direct conv: for each 5-row output block, 25 accumulating
fp32r matmuls (contraction over C=128 in partitions) into one PSUM bank.
The box-filter term conv(sigma_x + mu_x^2, ones[1,C,5,5]) is computed
cheaply per image: channel-sum via ones-matmul -> [96,96] plane, vertical
5-box via a banded matmul, horizontal 5-box on the vector engine; the
per-output-channel softplus scale is folded in as one extra rank-1 matmul
accumulated into each sigma PSUM group.  The 1e-3 scale is folded into
W^2 and softplus(w_sigma) host-side; bias is added during PSUM eviction
on the scalar engine.
"""

import numpy as np

import concourse.bacc as bacc
import concourse.tile as tile
from concourse import mybir
from concourse.bass_utils import run_bass_kernel_spmd

F32 = mybir.dt.float32
F32R = mybir.dt.float32r
BF16 = mybir.dt.bfloat16
FP16 = mybir.dt.float16

B, C, O, H, W_IN, KK = 16, 128, 256, 96, 96, 5
HO = WO = 92
NCORES = 8
BPC = B // NCORES          # images per core
OCH = O // 128             # output-channel chunks
RB = 5                     # output rows per PSUM group
NPIX = RB * WO             # 460 <= 512 (one fp32 PSUM bank)

# 19 output row blocks; the last starts at 87 so it stays full-height
# (rows 87..91), overlapping rows 87..89 of the previous block (benign
# double-write of identical values).  Full N=460 keeps fp32r at rate 1.
BLOCK_STARTS = [5 * i for i in range(18)] + [HO - RB]
# channel-sum chunks over the 96 input rows: 19 x 5 rows + one final
# 5-row chunk starting at 91 (rows 91..95, overlap rows 91..94).
CS_STARTS = [5 * i for i in range(19)] + [H - RB]
# row-block sets: all blocks in a set accumulate concurrently in distinct
# PSUM banks so one LDWEIGHTS serves the whole set (5x fewer weight loads)
BLOCK_SETS = [BLOCK_STARTS[i : i + 5] for i in range(0, len(BLOCK_STARTS), 5)]

_CACHE = {}


def _build(iters=1):
    key = ("nc", iters)
    if key in _CACHE:
        return _CACHE[key]

    nc = bacc.Bacc(None)
    mu_d = nc.dram_tensor("mu", [BPC, C, H, W_IN], F32R, kind="ExternalInput")
    sg_d = nc.dram_tensor("sg", [BPC, C, H, W_IN], F32R, kind="ExternalInput")
    wmu_d = nc.dram_tensor("wmu", [C, OCH, KK * KK, 128], F32R, kind="ExternalInput")
    wsg_d = nc.dram_tensor("wsg", [C, OCH, KK * KK, 128], F32R, kind="ExternalInput")
    bias_d = nc.dram_tensor("bias", [128, OCH], F32, kind="ExternalInput")
    sp_d = nc.dram_tensor("sp", [1, O], F32, kind="ExternalInput")
    band_d = nc.dram_tensor("band", [H, HO], F32, kind="ExternalInput")
    muy_d = nc.dram_tensor("muy", [BPC, O, HO, WO], F32, kind="ExternalOutput")
    sgy_d = nc.dram_tensor("sgy", [BPC, O, HO, WO], F32, kind="ExternalOutput")

    with tile.TileContext(nc) as tc:
        with (
            tc.tile_pool(name="consts", bufs=1) as consts,
            tc.tile_pool(name="imgs", bufs=1) as imgs,
            tc.tile_pool(name="boxs", bufs=2) as boxs,
            tc.tile_pool(name="ufc", bufs=3) as ufc,
            tc.tile_pool(name="bfc", bufs=4) as bfc,
            tc.tile_pool(name="stag_mu", bufs=3) as stag_mu,
            tc.tile_pool(name="stag_sg", bufs=3) as stag_sg,
            tc.tile_pool(name="ps_conv", bufs=6, space="PSUM") as ps_conv,
            tc.tile_pool(name="ps_u", bufs=1, space="PSUM") as ps_u,
            tc.tile_pool(name="ps_v", bufs=1, space="PSUM") as ps_v,
        ):
            wmu_sb = consts.tile([C, OCH, KK * KK, 128], F32R)
            wsg_sb = consts.tile([C, OCH, KK * KK, 128], F32R)
            bias_sb = consts.tile([128, OCH], F32)
            sp_sb = consts.tile([1, O], F32)
            band_sb = consts.tile([H, HO], F32)
            ones_col = consts.tile([C, 1], FP16)
            nc.sync.dma_start(wmu_sb[:], wmu_d[:])
            nc.sync.dma_start(wsg_sb[:], wsg_d[:])
            nc.sync.dma_start(bias_sb[:], bias_d[:])
            nc.sync.dma_start(sp_sb[:], sp_d[:])
            nc.sync.dma_start(band_sb[:], band_d[:])
            nc.vector.memset(ones_col[:], 1.0)

            import contextlib

            loop_cm = tc.For_i(0, iters, 1) if iters > 1 else contextlib.nullcontext()
            with loop_cm:
              for img in range(BPC):
                  mu_sb = imgs.tile([C, H, W_IN], F32R, tag="mu")
                  sg_sb = imgs.tile([C, H, W_IN], F32R, tag="sg")
                  nc.sync.dma_start(mu_sb[:], mu_d[img])
                  nc.sync.dma_start(sg_sb[:], sg_d[img])

                  # t = sigma_x + mu_x^2 (bf16 is plenty: it only feeds the
                  # 3200-term box sums, where rounding error averages out)
                  t_bf = imgs.tile([C, H, W_IN], FP16, tag="t")
                  nc.vector.tensor_mul(t_bf[:], mu_sb[:].bitcast(F32), mu_sb[:].bitcast(F32))
                  nc.vector.tensor_add(t_bf[:], t_bf[:], sg_sb[:].bitcast(F32))

                  def conv_group(x_sb, w_sb, r0, och, extra, evict):
                      ps = ps_conv.tile([128, RB, WO], F32, tag="ps")
                      ki = 0
                      for kh in range(KK):
                          for kw in range(KK):
                              nc.tensor.matmul(
                                  ps[:],
                                  w_sb[:, och, ki, :],
                                  x_sb[:, r0 + kh : r0 + kh + RB, kw : kw + WO],
                                  start=(ki == 0),
                                  stop=False,
                              )
                              ki += 1
                      extra(ps)
                      evict(ps)

                  # ---- mu conv, first output-channel chunk ----
                  def mu_extra(ps, och):
                      # close the accumulation group: bias is added at
                      # eviction time on the scalar engine
                      pass

                  def mu_evict(ps, r0, och):
                      st = stag_mu.tile([128, RB, WO], F32, tag="st")
                      nc.scalar.add(st[:], ps[:], bias_sb[:, och : och + 1])
                      nc.sync.dma_start(
                          muy_d[img, och * 128 : (och + 1) * 128, r0 : r0 + RB, :],
                          st[:],
                      )

                  def sg_evict(ps, r0, och):
                      st = stag_sg.tile([128, RB, WO], F32, tag="st")
                      nc.vector.tensor_copy(st[:], ps[:])
                      nc.sync.dma_start(
                          sgy_d[img, och * 128 : (och + 1) * 128, r0 : r0 + RB, :],
                          st[:],
                      )

                  def mu_groups(och):
                      for blocks in BLOCK_SETS:
                          pss = [
                              (r0, ps_conv.tile([128, RB, WO], F32, tag="ps", name=f"ps{r0}"))
                              for r0 in blocks
                          ]
                          for ki in range(KK * KK):
                              kh, kw = divmod(ki, KK)
                              for r0, ps in pss:
                                  nc.tensor.matmul(
                                      ps[:],
                                      wmu_sb[:, och, ki, :],
                                      mu_sb[:, r0 + kh : r0 + kh + RB, kw : kw + WO],
                                      start=(ki == 0),
                                      stop=(ki == KK * KK - 1),
                                  )
                          for r0, ps in pss:
                              mu_evict(ps, r0, och)

                  mu_groups(0)

                  # ---- box-filter pipeline (between the two mu chunks so the
                  # PE never waits on it) ----
                  u2d = boxs.tile([H, W_IN], F32, tag="u2d")
                  for cs in CS_STARTS:
                      ups = ps_u.tile([1, RB * W_IN], F32, tag="ups")
                      nc.tensor.matmul(
                          ups[:],
                          ones_col[:],
                          t_bf[:, cs : cs + RB, :],
                          start=True,
                          stop=True,
                      )
                      uf = ufc.tile([1, RB * W_IN], F32, tag="uf")
                      nc.scalar.copy(uf[:], ups[:])
                      nc.sync.dma_start(u2d[cs : cs + RB, :], uf[:])

                  vb_ps = ps_v.tile([HO, W_IN], F32, tag="vb")
                  nc.tensor.matmul(
                      vb_ps[:],
                      band_sb[:],
                      u2d[:],
                      start=True,
                      stop=True,
                  )
                  vb_sb = boxs.tile([HO, W_IN], F32, tag="vbs")
                  nc.vector.tensor_copy(vb_sb[:], vb_ps[:])
                  box2d = boxs.tile([HO, WO], F32, tag="box")
                  nc.vector.tensor_add(box2d[:], vb_sb[:, 0:WO], vb_sb[:, 1 : 1 + WO])
                  for kw in (2, 3, 4):
                      nc.vector.tensor_add(box2d[:], box2d[:], vb_sb[:, kw : kw + WO])

                  mu_groups(1)

                  # ---- sigma conv (both chunks); the softplus term enters as
                  # one rank-1 matmul accumulated into the same PSUM group ----
                  for och in range(OCH):
                      for blocks in BLOCK_SETS:
                          bfs = []
                          for r0 in blocks:
                              bf = bfc.tile([1, RB * WO], F32, tag="bf", name=f"bf{r0}")
                              nc.sync.dma_start(bf[:], box2d[r0 : r0 + RB, :])
                              bfs.append(bf)
                          pss = [
                              (r0, ps_conv.tile([128, RB, WO], F32, tag="ps", name=f"ps{r0}"))
                              for r0 in blocks
                          ]
                          for ki in range(KK * KK):
                              kh, kw = divmod(ki, KK)
                              for r0, ps in pss:
                                  nc.tensor.matmul(
                                      ps[:],
                                      wsg_sb[:, och, ki, :],
                                      sg_sb[:, r0 + kh : r0 + kh + RB, kw : kw + WO],
                                      start=(ki == 0),
                                      stop=False,
                                  )
                          for (r0, ps), bf in zip(pss, bfs):
                              nc.tensor.matmul(
                                  ps[:],
                                  sp_sb[0:1, och * 128 : (och + 1) * 128],
                                  bf[:],
                                  start=False,
                                  stop=True,
                              )
                          for r0, ps in pss:
                              sg_evict(ps, r0, och)

    nc.compile()
    _CACHE[key] = nc
    return nc


def _host_prep(mu_x, sigma_x, W, bias, w_sigma):
    W = np.asarray(W, dtype=np.float32)
    bias = np.asarray(bias, dtype=np.float32)
    w_sigma = np.asarray(w_sigma, dtype=np.float32)

    # [o, c, kh, kw] -> [c, och, k, o_in]
    w4 = W.reshape(OCH, 128, C, KK * KK)
    wmu = np.ascontiguousarray(w4.transpose(2, 0, 3, 1))
    w2 = (W.astype(np.float64) ** 2 * 1e-3).astype(np.float32)
    wsg = np.ascontiguousarray(
        w2.reshape(OCH, 128, C, KK * KK).transpose(2, 0, 3, 1)
    )
    bias_arr = np.ascontiguousarray(bias.reshape(OCH, 128).T)
    sp = np.log(1.0 + np.exp(np.maximum(w_sigma.astype(np.float64), -88.0)))
    sp_row = np.ascontiguousarray((sp[:, 0] * 1e-3).astype(np.float32)[None, :])
    band = np.zeros((H, HO), dtype=np.float32)
    for y2 in range(HO):
        band[y2 : y2 + KK, y2] = 1.0
    return wmu, wsg, bias_arr, sp_row, band


def kernel(mu_x, sigma_x, W, bias, w_sigma):
    mu_x = np.asarray(mu_x, dtype=np.float32)
    sigma_x = np.asarray(sigma_x, dtype=np.float32)
    wmu, wsg, bias_arr, sp_row, band = _host_prep(mu_x, sigma_x, W, bias, w_sigma)

    nc = _build()
    in_maps = []
    for c in range(NCORES):
        in_maps.append(
            {
                "mu": mu_x[c * BPC : (c + 1) * BPC],
                "sg": sigma_x[c * BPC : (c + 1) * BPC],
                "wmu": wmu,
                "wsg": wsg,
                "bias": bias_arr,
                "sp": sp_row,
                "band": band,
            }
        )
    res = run_bass_kernel_spmd(nc, in_maps, core_ids=list(range(NCORES)))
    mu_y = np.concatenate([res.results[c]["muy"] for c in range(NCORES)], axis=0)
    sigma_y = np.concatenate([res.results[c]["sgy"] for c in range(NCORES)], axis=0)
    return mu_y.astype(np.float32), sigma_y.astype(np.float32)



# revision 3
# speedup vs baseline: 1.9041x; 1.9041x over previous
"""Trainium2 Bass kernel for the mu/sigma Conv2d problem.

Math (per reference):
  mu_y    = conv(mu_x, W) + bias
  sigma_y = (softplus(w_sigma) * (conv(sigma_x, ones) + conv(mu_x^2, ones))
             + conv(sigma_x, W^2)) * 1e-3

Shapes: mu_x/sigma_x [16,128,96,96], W [256,128,5,5], bias [256],
w_sigma [256,1].  Outputs [16,256,92,92] (VALID conv).

Strategy: data-parallel over batch across 8 NeuronCores (2 images/core).
Direct conv, block-major: each 5-row output block accumulates its taps
back-to-back into one PSUM bank while the previous block's eviction
overlaps on the scalar/vector engines.

Precision plan (gate is 2e-2 relative to max):
  - mu conv in bf16 (weights + moving): FWL weight loads hide in the PE
    reorder window; measured ~211 ns per N=460 matmul.
  - sigma conv conv(sigma_x, W^2) in fp8 e4m3 with perf_mode=DoubleRow:
    taps are paired into the two fp8 k-tiles (row pairs via the image's
    96-byte row stride, the kh=4 row via a 1-column-shifted second copy
    of the image), 12 pair-MMs + 1 single per block instead of 25.
    W^2 is pre-scaled by a host-chosen power of two `a` so it sits in
    e4m3 range; the 1e-3/a dequant rides the eviction multiply.  This
    term is ~3 orders of magnitude below sigma_y's box-filter term, so
    fp8 error is invisible.
  - box-filter term: t = mu^2 + sigma_x in e4m3; channel sums via
    DoubleRow selector matmuls (two 5-row chunks per matmul), vertical
    5-box via a banded fp32 matmul, horizontal 5-box on the vector
    engine; enters each sigma PSUM group as one fp16 rank-1 matmul
    (softplus row (x) box row).  A factor 8 rides the selector weights
    so the fp16 softplus row stays in range.
"""

import numpy as np
import ml_dtypes

import concourse.bacc as bacc
import concourse.tile as tile
from concourse import mybir
from concourse.bass_utils import run_bass_kernel_spmd

F32 = mybir.dt.float32
F16 = mybir.dt.float16
BF16 = mybir.dt.bfloat16
F8 = mybir.dt.float8e4

B, C, O, H, W_IN, KK = 16, 128, 256, 96, 96, 5
HO = WO = 92
NCORES = 8
BPC = B // NCORES          # images per core
OCH = O // 128             # output-channel chunks
RB = 5                     # output rows per PSUM block
NPIX = RB * WO             # 460 <= 512 (one fp32 PSUM bank)

# 19 output row blocks; the last starts at 87 so it stays full-height
# (rows 87..91), overlapping rows 87..89 of the previous block (benign
# double-write of identical values).
BLOCK_STARTS = [5 * i for i in range(18)] + [HO - RB]
# channel-sum row-chunk pairs over the 96 input rows; each DoubleRow
# selector matmul sums two 5-row chunks at once.  The last pair (90,91)
# overlaps rows 91..94 (benign double-write of identical values).
CS_PAIRS = [(10 * i, 10 * i + 5) for i in range(9)] + [(90, 91)]

_CACHE = {}


def _pair(base, stride):
    """Handcraft the DoubleRow k-tile dim: [P, 2, ...] with the given
    element stride for the pair dimension (overlapping reads are fine)."""
    v = base.copy()
    a = v.ap
    lst = a.to_list()
    a.clear()
    a.extend([lst[0], [int(stride), 2]] + lst[1:])
    return v


def _build(iters=1):
    key = ("nc", iters)
    if key in _CACHE:
        return _CACHE[key]

    DR = mybir.MatmulPerfMode.DoubleRow

    nc = bacc.Bacc(None)
    mu_d = nc.dram_tensor("mu", [BPC, C, H, W_IN], BF16, kind="ExternalInput")
    sg_d = nc.dram_tensor("sg", [BPC, C, H, W_IN], F8, kind="ExternalInput")
    wmu_d = nc.dram_tensor("wmu", [C, OCH, KK * KK, 128], BF16, kind="ExternalInput")
    wsg_d = nc.dram_tensor("wsg", [C, OCH, KK * KK, 128], F8, kind="ExternalInput")
    bias_d = nc.dram_tensor("bias", [128, OCH], F32, kind="ExternalInput")
    spx_d = nc.dram_tensor("spx", [1, O], F16, kind="ExternalInput")
    sel_d = nc.dram_tensor("sel", [C, 2, 16], F8, kind="ExternalInput")
    band_d = nc.dram_tensor("band", [H, HO], F32, kind="ExternalInput")
    sout_d = nc.dram_tensor("sout", [128, 1], F32, kind="ExternalInput")
    muy_d = nc.dram_tensor("muy", [BPC, O, HO, WO], F32, kind="ExternalOutput")
    sgy_d = nc.dram_tensor("sgy", [BPC, O, HO, WO], F32, kind="ExternalOutput")

    with tile.TileContext(nc) as tc:
        with (
            tc.tile_pool(name="consts", bufs=1) as consts,
            tc.tile_pool(name="imgs", bufs=2) as imgs,
            tc.tile_pool(name="boxs", bufs=2) as boxs,
            tc.tile_pool(name="ufc", bufs=2) as ufc,
            tc.tile_pool(name="bfc", bufs=3) as bfc,
            tc.tile_pool(name="stag_mu", bufs=3) as stag_mu,
            tc.tile_pool(name="stag_sg", bufs=3) as stag_sg,
            tc.tile_pool(name="ps_conv", bufs=4, space="PSUM") as ps_conv,
            tc.tile_pool(name="ps_u", bufs=2, space="PSUM") as ps_u,
            tc.tile_pool(name="ps_v", bufs=1, space="PSUM") as ps_v,
        ):
            wmu_sb = consts.tile([C, OCH, KK * KK, 128], BF16)
            wsg_sb = consts.tile([C, OCH, KK * KK, 128], F8)
            bias_sb = consts.tile([128, OCH], F32)
            spx_sb = consts.tile([1, O], F16)
            sel_sb = consts.tile([C, 2, 16], F8)
            band_sb = consts.tile([H, HO], F32)
            sout_sb = consts.tile([128, 1], F32)
            nc.sync.dma_start(wmu_sb[:], wmu_d[:])
            nc.sync.dma_start(wsg_sb[:], wsg_d[:])
            nc.sync.dma_start(bias_sb[:], bias_d[:])
            nc.sync.dma_start(spx_sb[:], spx_d[:])
            nc.sync.dma_start(sel_sb[:], sel_d[:])
            nc.sync.dma_start(band_sb[:], band_d[:])
            nc.sync.dma_start(sout_sb[:], sout_d[:])

            import contextlib

            loop_cm = tc.For_i(0, iters, 1) if iters > 1 else contextlib.nullcontext()
            with loop_cm:
              for img in range(BPC):
                  mu_sb = imgs.tile([C, H, W_IN], BF16, tag="mu")
                  # dim1: copy 0 = sigma, copy 1 = sigma shifted one column
                  # left (for pairing the kh=4 taps across kw)
                  sg_sb = imgs.tile([C, 2, H, W_IN], F8, tag="sg")
                  nc.sync.dma_start(mu_sb[:], mu_d[img])
                  nc.sync.dma_start(sg_sb[:, 0], sg_d[img])
                  nc.sync.dma_start(
                      sg_sb[:, 1, :, 0 : W_IN - 1], sg_d[img, :, :, 1:W_IN]
                  )

                  # t = mu^2 + sigma in e4m3 (feeds only the 3200-term box
                  # sums, where quantization noise averages out)
                  t_sb = imgs.tile([C, H, W_IN], F8, tag="t")
                  nc.vector.tensor_mul(t_sb[:], mu_sb[:], mu_sb[:])
                  nc.vector.tensor_add(t_sb[:], t_sb[:], sg_sb[:, 0])

                  u2d = boxs.tile([H, W_IN], F32, tag="u2d")
                  box16 = boxs.tile([HO, WO], F16, tag="box16")

                  def ones_mm(ci):
                      cs0, cs1 = CS_PAIRS[ci]
                      ups = ps_u.tile([16, 2 * RB * W_IN // 2], F32, tag="ups")
                      tv = _pair(t_sb[:, cs0 : cs0 + RB, :], (cs1 - cs0) * W_IN)
                      nc.tensor.matmul(
                          ups[:], sel_sb[:], tv, start=True, stop=True,
                          perf_mode=DR,
                      )
                      uf = ufc.tile([2, RB * W_IN], F32, tag="uf")
                      nc.scalar.copy(uf[:], ups[0:2])
                      if cs1 == cs0 + RB:
                          nc.sync.dma_start(u2d[cs0 : cs0 + 2 * RB, :], uf[:])
                      else:
                          nc.sync.dma_start(u2d[cs0 : cs0 + RB, :], uf[0:1])
                          nc.sync.dma_start(u2d[cs1 : cs1 + RB, :], uf[1:2])

                  def band_mm():
                      vb_ps = ps_v.tile([HO, W_IN], F32, tag="vb")
                      nc.tensor.matmul(
                          vb_ps[:], band_sb[:], u2d[:], start=True, stop=True
                      )
                      vb_sb = boxs.tile([HO, W_IN], F32, tag="vbs")
                      nc.vector.tensor_copy(vb_sb[:], vb_ps[:])
                      box32 = boxs.tile([HO, WO], F32, tag="box32")
                      nc.vector.tensor_add(
                          box32[:], vb_sb[:, 0:WO], vb_sb[:, 1 : 1 + WO]
                      )
                      for kw in (2, 3, 4):
                          nc.vector.tensor_add(
                              box32[:], box32[:], vb_sb[:, kw : kw + WO]
                          )
                      nc.vector.tensor_copy(box16[:], box32[:])

                  # PE-side extras injected between mu conv blocks so the
                  # scalar-engine ups evictions never stall the PE
                  extras = [lambda ci=ci: ones_mm(ci) for ci in range(len(CS_PAIRS))]
                  extras += [None, band_mm]

                  def mu_chunk(och, extras=()):
                      for bi, r0 in enumerate(BLOCK_STARTS):
                          ps = ps_conv.tile([128, RB, WO], F32, tag="ps")
                          for ki in range(KK * KK):
                              kh, kw = divmod(ki, KK)
                              nc.tensor.matmul(
                                  ps[:],
                                  wmu_sb[:, och, ki, :],
                                  mu_sb[:, r0 + kh : r0 + kh + RB, kw : kw + WO],
                                  start=(ki == 0),
                                  stop=(ki == KK * KK - 1),
                              )
                          if bi < len(extras) and extras[bi] is not None:
                              extras[bi]()
                          st = stag_mu.tile([128, RB, WO], F32, tag="st")
                          nc.scalar.add(st[:], ps[:], bias_sb[:, och : och + 1])
                          nc.sync.dma_start(
                              muy_d[img, och * 128 : (och + 1) * 128, r0 : r0 + RB, :],
                              st[:],
                          )

                  def sg_chunk(och):
                      # per block: 10 row pairs (kh in {0,1},{2,3} x kw),
                      # 2 column pairs ((4,0)+(4,1), (4,2)+(4,3) via the
                      # shifted copy), 1 single (4,4), then the fp16
                      # rank-1 box term closes the accumulation group.
                      for r0 in BLOCK_STARTS:
                          bf = bfc.tile([1, RB * WO], F16, tag="bf")
                          nc.sync.dma_start(bf[:], box16[r0 : r0 + RB, :])
                          ps = ps_conv.tile([128, RB, WO], F32, tag="ps")
                          first = True
                          for kw in range(KK):
                              for kh in (0, 2):
                                  ki = kh * KK + kw
                                  wv = _pair(wsg_sb[:, och, ki, :], KK * 128)
                                  xv = _pair(
                                      sg_sb[:, 0, r0 + kh : r0 + kh + RB, kw : kw + WO],
                                      W_IN,
                                  )
                                  nc.tensor.matmul(
                                      ps[:], wv, xv, start=first, stop=False,
                                      perf_mode=DR,
                                  )
                                  first = False
                          for kw in (0, 2):
                              ki = 4 * KK + kw
                              wv = _pair(wsg_sb[:, och, ki, :], 128)
                              xv = _pair(
                                  sg_sb[:, 0, r0 + 4 : r0 + 4 + RB, kw : kw + WO],
                                  H * W_IN,
                              )
                              nc.tensor.matmul(
                                  ps[:], wv, xv, start=False, stop=False,
                                  perf_mode=DR,
                              )
                          ki = 4 * KK + 4
                          nc.tensor.matmul(
                              ps[:],
                              wsg_sb[:, och, ki, :],
                              sg_sb[:, 0, r0 + 4 : r0 + 4 + RB, 4 : 4 + WO],
                              start=False,
                              stop=False,
                          )
                          nc.tensor.matmul(
                              ps[:],
                              spx_sb[0:1, och * 128 : (och + 1) * 128],
                              bf[:],
                              start=False,
                              stop=True,
                          )
                          st = stag_sg.tile([128, RB, WO], F32, tag="st")
                          nc.vector.tensor_scalar_mul(st[:], ps[:], sout_sb[:, 0:1])
                          nc.sync.dma_start(
                              sgy_d[img, och * 128 : (och + 1) * 128, r0 : r0 + RB, :],
                              st[:],
                          )

                  mu_chunk(0)
                  mu_chunk(1, extras)
                  sg_chunk(0)
                  sg_chunk(1)

    nc.compile()
    _CACHE[key] = nc
    return nc


def _host_prep(W, bias, w_sigma):
    W = np.asarray(W, dtype=np.float64)
    bias = np.asarray(bias, dtype=np.float32)
    w_sigma = np.asarray(w_sigma, dtype=np.float64)

    w2 = W * W
    w2max = float(w2.max())
    sp = np.log1p(np.exp(np.minimum(np.maximum(w_sigma, -88.0), 80.0)))
    sp = sp + np.maximum(w_sigma - 80.0, 0.0)  # softplus(x)~x for huge x
    spmax = float(sp.max())
    a_pow = int(np.floor(np.log2(224.0 / max(w2max, 1e-300))))
    a_pow = min(a_pow, int(np.floor(np.log2(60000.0 * 8.0 / max(spmax, 1e-300)))))
    a_pow = max(min(a_pow, 120), -120)
    a = float(2.0**a_pow)

    # [o, c, kh, kw] -> [c, och, ki, o_in]
    w4 = W.reshape(OCH, 128, C, KK * KK)
    wmu = np.ascontiguousarray(w4.transpose(2, 0, 3, 1)).astype(ml_dtypes.bfloat16)
    wsg = np.ascontiguousarray(
        (w2 * a).reshape(OCH, 128, C, KK * KK).transpose(2, 0, 3, 1)
    ).astype(ml_dtypes.float8_e4m3)
    bias_arr = np.ascontiguousarray(bias.reshape(OCH, 128).T)
    spx = (sp[:, 0] * (a / 8.0)).astype(np.float16)[None, :]
    spx = np.ascontiguousarray(spx)
    sout = np.full((128, 1), 1e-3 / a, dtype=np.float32)
    band = np.zeros((H, HO), dtype=np.float32)
    for y2 in range(HO):
        band[y2 : y2 + KK, y2] = 1.0
    sel = np.zeros((C, 2, 16), dtype=ml_dtypes.float8_e4m3)
    sel[:, 0, 0] = 8.0
    sel[:, 1, 1] = 8.0
    return wmu, wsg, bias_arr, spx, sout, band, sel


def kernel(mu_x, sigma_x, W, bias, w_sigma):
    mu_x = np.asarray(mu_x, dtype=np.float32).astype(ml_dtypes.bfloat16)
    sigma_x = np.asarray(sigma_x, dtype=np.float32).astype(ml_dtypes.float8_e4m3)
    wmu, wsg, bias_arr, spx, sout, band, sel = _host_prep(W, bias, w_sigma)

    nc = _build()
    in_maps = []
    for c in range(NCORES):
        in_maps.append(
            {
                "mu": mu_x[c * BPC : (c + 1) * BPC],
                "sg": sigma_x[c * BPC : (c + 1) * BPC],
                "wmu": wmu,
                "wsg": wsg,
                "bias": bias_arr,
                "spx": spx,
                "sel": sel,
                "band": band,
                "sout": sout,
            }
        )
    res = run_bass_kernel_spmd(nc, in_maps, core_ids=list(range(NCORES)))
    mu_y = np.concatenate([res.results[c]["muy"] for c in range(NCORES)], axis=0)
    sigma_y = np.concatenate([res.results[c]["sgy"] for c in range(NCORES)], axis=0)
    return mu_y.astype(np.float32), sigma_y.astype(np.float32)


# revision 5
# speedup vs baseline: 1.9056x; 1.0008x over previous
"""Trainium2 Bass kernel for the mu/sigma Conv2d problem.

Math (per reference):
  mu_y    = conv(mu_x, W) + bias
  sigma_y = (softplus(w_sigma) * (conv(sigma_x, ones) + conv(mu_x^2, ones))
             + conv(sigma_x, W^2)) * 1e-3

Shapes: mu_x/sigma_x [16,128,96,96], W [256,128,5,5], bias [256],
w_sigma [256,1].  Outputs [16,256,92,92] (VALID conv).

Strategy: data-parallel over batch across 8 NeuronCores (2 images/core).
Direct conv, block-major: each 5-row output block accumulates its taps
back-to-back into one PSUM bank while the previous block's eviction
overlaps on the scalar/vector engines.

Precision plan (gate is 2e-2 relative to max):
  - mu conv in bf16 (weights + moving): FWL weight loads hide in the PE
    reorder window; measured ~211 ns per N=460 matmul.
  - sigma conv conv(sigma_x, W^2) in fp8 e4m3 with perf_mode=DoubleRow:
    taps are paired into the two fp8 k-tiles (row pairs via the image's
    96-byte row stride, the kh=4 row via a 1-column-shifted second copy
    of the image), 12 pair-MMs + 1 single per block instead of 25.
    W^2 is pre-scaled by a host-chosen power of two `a` so it sits in
    e4m3 range; the 1e-3/a dequant rides the eviction multiply.  This
    term is ~3 orders of magnitude below sigma_y's box-filter term, so
    fp8 error is invisible.
  - box-filter term: t = mu^2 + sigma_x in e4m3; channel sums via
    DoubleRow selector matmuls (two 5-row chunks per matmul), vertical
    5-box via a banded fp32 matmul, horizontal 5-box on the vector
    engine; enters each sigma PSUM group as one fp16 rank-1 matmul
    (softplus row (x) box row).  A factor 8 rides the selector weights
    so the fp16 softplus row stays in range.
"""

import numpy as np
import ml_dtypes

import concourse.bacc as bacc
import concourse.tile as tile
from concourse import mybir
from concourse.bass_utils import run_bass_kernel_spmd

F32 = mybir.dt.float32
F16 = mybir.dt.float16
BF16 = mybir.dt.bfloat16
F8 = mybir.dt.float8e4

B, C, O, H, W_IN, KK = 16, 128, 256, 96, 96, 5
HO = WO = 92
NCORES = 8
BPC = B // NCORES          # images per core
OCH = O // 128             # output-channel chunks
RB = 5                     # output rows per PSUM block
NPIX = RB * WO             # 460 <= 512 (one fp32 PSUM bank)

# 19 output row blocks; the last starts at 87 so it stays full-height
# (rows 87..91), overlapping rows 87..89 of the previous block (benign
# double-write of identical values).
BLOCK_STARTS = [5 * i for i in range(18)] + [HO - RB]
# channel-sum row-chunk pairs over the 96 input rows; each DoubleRow
# selector matmul sums two 5-row chunks at once.  The last pair (90,91)
# overlaps rows 91..94 (benign double-write of identical values).
CS_PAIRS = [(10 * i, 10 * i + 5) for i in range(9)] + [(90, 91)]

_CACHE = {}


def _pair(base, stride):
    """Handcraft the DoubleRow k-tile dim: [P, 2, ...] with the given
    element stride for the pair dimension (overlapping reads are fine)."""
    v = base.copy()
    a = v.ap
    lst = a.to_list()
    a.clear()
    a.extend([lst[0], [int(stride), 2]] + lst[1:])
    return v


def _build(iters=1):
    key = ("nc", iters)
    if key in _CACHE:
        return _CACHE[key]

    DR = mybir.MatmulPerfMode.DoubleRow

    nc = bacc.Bacc(None)
    mu_d = nc.dram_tensor("mu", [BPC, C, H, W_IN], BF16, kind="ExternalInput")
    sg_d = nc.dram_tensor("sg", [BPC, C, H, W_IN], F8, kind="ExternalInput")
    wmu_d = nc.dram_tensor("wmu", [C, OCH, KK * KK, 128], BF16, kind="ExternalInput")
    wsg_d = nc.dram_tensor("wsg", [C, OCH, KK * KK, 128], F8, kind="ExternalInput")
    bias_d = nc.dram_tensor("bias", [128, OCH], F32, kind="ExternalInput")
    spx_d = nc.dram_tensor("spx", [1, O], F16, kind="ExternalInput")
    sel_d = nc.dram_tensor("sel", [C, 2, 16], F8, kind="ExternalInput")
    band_d = nc.dram_tensor("band", [H, HO], F32, kind="ExternalInput")
    sout_d = nc.dram_tensor("sout", [128, 1], F32, kind="ExternalInput")
    muy_d = nc.dram_tensor("muy", [BPC, O, HO, WO], F32, kind="ExternalOutput")
    sgy_d = nc.dram_tensor("sgy", [BPC, O, HO, WO], F32, kind="ExternalOutput")

    with tile.TileContext(nc) as tc:
        with (
            tc.tile_pool(name="consts", bufs=1) as consts,
            tc.tile_pool(name="imgs", bufs=2) as imgs,
            tc.tile_pool(name="boxs", bufs=2) as boxs,
            tc.tile_pool(name="ufc", bufs=2) as ufc,
            tc.tile_pool(name="bfc", bufs=3) as bfc,
            tc.tile_pool(name="stag_mu", bufs=3) as stag_mu,
            tc.tile_pool(name="stag_sg", bufs=3) as stag_sg,
            tc.tile_pool(name="ps_conv", bufs=5, space="PSUM") as ps_conv,
            tc.tile_pool(name="ps_u", bufs=2, space="PSUM") as ps_u,
            tc.tile_pool(name="ps_v", bufs=1, space="PSUM") as ps_v,
        ):
            wmu_sb = consts.tile([C, OCH, KK * KK, 128], BF16)
            wsg_sb = consts.tile([C, OCH, KK * KK, 128], F8)
            bias_sb = consts.tile([128, OCH], F32)
            spx_sb = consts.tile([1, O], F16)
            sel_sb = consts.tile([C, 2, 16], F8)
            band_sb = consts.tile([H, HO], F32)
            sout_sb = consts.tile([128, 1], F32)
            nc.sync.dma_start(wmu_sb[:], wmu_d[:])
            nc.sync.dma_start(wsg_sb[:], wsg_d[:])
            nc.sync.dma_start(bias_sb[:], bias_d[:])
            nc.sync.dma_start(spx_sb[:], spx_d[:])
            nc.sync.dma_start(sel_sb[:], sel_d[:])
            nc.sync.dma_start(band_sb[:], band_d[:])
            nc.sync.dma_start(sout_sb[:], sout_d[:])

            import contextlib

            loop_cm = tc.For_i(0, iters, 1) if iters > 1 else contextlib.nullcontext()
            with loop_cm:
              for img in range(BPC):
                  mu_sb = imgs.tile([C, H, W_IN], BF16, tag="mu")
                  # dim1: copy 0 = sigma, copy 1 = sigma shifted one column
                  # left (for pairing the kh=4 taps across kw)
                  sg_sb = imgs.tile([C, 2, H, W_IN], F8, tag="sg")
                  # input DMAs ride the Activation HWDGE queue so they can
                  # prefetch across the loop/image boundary instead of
                  # queuing behind the output DMAs on the SP queue
                  nc.scalar.dma_start(mu_sb[:], mu_d[img])
                  nc.scalar.dma_start(sg_sb[:, 0], sg_d[img])
                  nc.scalar.dma_start(
                      sg_sb[:, 1, :, 0 : W_IN - 1], sg_d[img, :, :, 1:W_IN]
                  )

                  # t = mu^2 + sigma in e4m3 (feeds only the 3200-term box
                  # sums, where quantization noise averages out)
                  t_sb = imgs.tile([C, H, W_IN], F8, tag="t")
                  nc.vector.tensor_mul(t_sb[:], mu_sb[:], mu_sb[:])
                  nc.vector.tensor_add(t_sb[:], t_sb[:], sg_sb[:, 0])

                  u2d = boxs.tile([H, W_IN], F32, tag="u2d")
                  box16 = boxs.tile([HO, WO], F16, tag="box16")

                  def ones_mm(ci):
                      cs0, cs1 = CS_PAIRS[ci]
                      ups = ps_u.tile([16, 2 * RB * W_IN // 2], F32, tag="ups")
                      tv = _pair(t_sb[:, cs0 : cs0 + RB, :], (cs1 - cs0) * W_IN)
                      nc.tensor.matmul(
                          ups[:], sel_sb[:], tv, start=True, stop=True,
                          perf_mode=DR,
                      )
                      uf = ufc.tile([2, RB * W_IN], F32, tag="uf")
                      nc.scalar.copy(uf[:], ups[0:2])
                      if cs1 == cs0 + RB:
                          nc.sync.dma_start(u2d[cs0 : cs0 + 2 * RB, :], uf[:])
                      else:
                          nc.sync.dma_start(u2d[cs0 : cs0 + RB, :], uf[0:1])
                          nc.sync.dma_start(u2d[cs1 : cs1 + RB, :], uf[1:2])

                  def band_mm():
                      vb_ps = ps_v.tile([HO, W_IN], F32, tag="vb")
                      nc.tensor.matmul(
                          vb_ps[:], band_sb[:], u2d[:], start=True, stop=True
                      )
                      vb_sb = boxs.tile([HO, W_IN], F32, tag="vbs")
                      nc.vector.tensor_copy(vb_sb[:], vb_ps[:])
                      box32 = boxs.tile([HO, WO], F32, tag="box32")
                      nc.vector.tensor_add(
                          box32[:], vb_sb[:, 0:WO], vb_sb[:, 1 : 1 + WO]
                      )
                      for kw in (2, 3, 4):
                          nc.vector.tensor_add(
                              box32[:], box32[:], vb_sb[:, kw : kw + WO]
                          )
                      nc.vector.tensor_copy(box16[:], box32[:])

                  # PE-side extras injected between mu conv blocks so the
                  # scalar-engine ups evictions never stall the PE
                  extras = [lambda ci=ci: ones_mm(ci) for ci in range(len(CS_PAIRS))]
                  extras += [None, band_mm]

                  def mu_chunk(och, extras=()):
                      for bi, r0 in enumerate(BLOCK_STARTS):
                          ps = ps_conv.tile([128, RB, WO], F32, tag="ps")
                          for ki in range(KK * KK):
                              kh, kw = divmod(ki, KK)
                              nc.tensor.matmul(
                                  ps[:],
                                  wmu_sb[:, och, ki, :],
                                  mu_sb[:, r0 + kh : r0 + kh + RB, kw : kw + WO],
                                  start=(ki == 0),
                                  stop=(ki == KK * KK - 1),
                              )
                          if bi < len(extras) and extras[bi] is not None:
                              extras[bi]()
                          st = stag_mu.tile([128, RB, WO], F32, tag="st")
                          nc.scalar.add(st[:], ps[:], bias_sb[:, och : och + 1])
                          nc.sync.dma_start(
                              muy_d[img, och * 128 : (och + 1) * 128, r0 : r0 + RB, :],
                              st[:],
                          )

                  def sg_chunk(och):
                      # per block: 10 row pairs (kh in {0,1},{2,3} x kw),
                      # 2 column pairs ((4,0)+(4,1), (4,2)+(4,3) via the
                      # shifted copy), 1 single (4,4), then the fp16
                      # rank-1 box term closes the accumulation group.
                      for r0 in BLOCK_STARTS:
                          bf = bfc.tile([1, RB * WO], F16, tag="bf")
                          nc.sync.dma_start(bf[:], box16[r0 : r0 + RB, :])
                          ps = ps_conv.tile([128, RB, WO], F32, tag="ps")
                          first = True
                          for kw in range(KK):
                              for kh in (0, 2):
                                  ki = kh * KK + kw
                                  wv = _pair(wsg_sb[:, och, ki, :], KK * 128)
                                  xv = _pair(
                                      sg_sb[:, 0, r0 + kh : r0 + kh + RB, kw : kw + WO],
                                      W_IN,
                                  )
                                  nc.tensor.matmul(
                                      ps[:], wv, xv, start=first, stop=False,
                                      perf_mode=DR,
                                  )
                                  first = False
                          for kw in (0, 2):
                              ki = 4 * KK + kw
                              wv = _pair(wsg_sb[:, och, ki, :], 128)
                              xv = _pair(
                                  sg_sb[:, 0, r0 + 4 : r0 + 4 + RB, kw : kw + WO],
                                  H * W_IN,
                              )
                              nc.tensor.matmul(
                                  ps[:], wv, xv, start=False, stop=False,
                                  perf_mode=DR,
                              )
                          ki = 4 * KK + 4
                          nc.tensor.matmul(
                              ps[:],
                              wsg_sb[:, och, ki, :],
                              sg_sb[:, 0, r0 + 4 : r0 + 4 + RB, 4 : 4 + WO],
                              start=False,
                              stop=False,
                          )
                          nc.tensor.matmul(
                              ps[:],
                              spx_sb[0:1, och * 128 : (och + 1) * 128],
                              bf[:],
                              start=False,
                              stop=True,
                          )
                          st = stag_sg.tile([128, RB, WO], F32, tag="st")
                          nc.vector.tensor_scalar_mul(st[:], ps[:], sout_sb[:, 0:1])
                          nc.sync.dma_start(
                              sgy_d[img, och * 128 : (och + 1) * 128, r0 : r0 + RB, :],
                              st[:],
                          )

                  mu_chunk(0)
                  mu_chunk(1, extras)
                  sg_chunk(0)
                  sg_chunk(1)

    nc.compile()
    _CACHE[key] = nc
    return nc


def _host_prep(W, bias, w_sigma):
    W = np.asarray(W, dtype=np.float64)
    bias = np.asarray(bias, dtype=np.float32)
    w_sigma = np.asarray(w_sigma, dtype=np.float64)

    w2 = W * W
    w2max = float(w2.max())
    sp = np.log1p(np.exp(np.minimum(np.maximum(w_sigma, -88.0), 80.0)))
    sp = sp + np.maximum(w_sigma - 80.0, 0.0)  # softplus(x)~x for huge x
    spmax = float(sp.max())
    a_pow = int(np.floor(np.log2(224.0 / max(w2max, 1e-300))))
    a_pow = min(a_pow, int(np.floor(np.log2(60000.0 * 8.0 / max(spmax, 1e-300)))))
    a_pow = max(min(a_pow, 120), -120)
    a = float(2.0**a_pow)

    # [o, c, kh, kw] -> [c, och, ki, o_in]
    w4 = W.reshape(OCH, 128, C, KK * KK)
    wmu = np.ascontiguousarray(w4.transpose(2, 0, 3, 1)).astype(ml_dtypes.bfloat16)
    wsg = np.ascontiguousarray(
        (w2 * a).reshape(OCH, 128, C, KK * KK).transpose(2, 0, 3, 1)
    ).astype(ml_dtypes.float8_e4m3)
    bias_arr = np.ascontiguousarray(bias.reshape(OCH, 128).T)
    spx = (sp[:, 0] * (a / 8.0)).astype(np.float16)[None, :]
    spx = np.ascontiguousarray(spx)
    sout = np.full((128, 1), 1e-3 / a, dtype=np.float32)
    band = np.zeros((H, HO), dtype=np.float32)
    for y2 in range(HO):
        band[y2 : y2 + KK, y2] = 1.0
    sel = np.zeros((C, 2, 16), dtype=ml_dtypes.float8_e4m3)
    sel[:, 0, 0] = 8.0
    sel[:, 1, 1] = 8.0
    return wmu, wsg, bias_arr, spx, sout, band, sel


def kernel(mu_x, sigma_x, W, bias, w_sigma):
    mu_x = np.asarray(mu_x, dtype=np.float32).astype(ml_dtypes.bfloat16)
    sigma_x = np.asarray(sigma_x, dtype=np.float32).astype(ml_dtypes.float8_e4m3)
    wmu, wsg, bias_arr, spx, sout, band, sel = _host_prep(W, bias, w_sigma)

    nc = _build()
    in_maps = []
    for c in range(NCORES):
        in_maps.append(
            {
                "mu": mu_x[c * BPC : (c + 1) * BPC],
                "sg": sigma_x[c * BPC : (c + 1) * BPC],
                "wmu": wmu,
                "wsg": wsg,
                "bias": bias_arr,
                "spx": spx,
                "sel": sel,
                "band": band,
                "sout": sout,
            }
        )
    res = run_bass_kernel_spmd(nc, in_maps, core_ids=list(range(NCORES)))
    mu_y = np.concatenate([res.results[c]["muy"] for c in range(NCORES)], axis=0)
    sigma_y = np.concatenate([res.results[c]["sgy"] for c in range(NCORES)], axis=0)
    return mu_y.astype(np.float32), sigma_y.astype(np.float32)
